# revision 27
# baseline (speedup 1.0000x reference)
"""EEND-SS loss device kernel (raw Bass, SPMD over 8 cores).

Device computes, per core (B_LOC=4 samples), the Gram matrix of
[sep rows(12) | src rows(12) | ones] over T, chunk-blocked so the host
extracts all pairwise dots / sums / sq-sums for the SI-SDR PIT loss.
sep+src are 97% of the input bytes (12.3 MB/core); the tiny diarization
BCE (diar_probs 48 KB + labels 384 KB per 4 samples) and existence BCE
are done host-side in numpy -- their scattered-granule DMAs would grind
the SDMA engines for ~30us, longer than the compute they feed.

Pipeline (T split into column-phases of widths[ph] chunks, default
(250, 250, 250, 170, 80)):
  DMA (sequential HBM)        ->  staging[ph]   [128, 24*W] f32
  repack (DVE/ACT row split)  ->  blocked[ph]   [128, 26*W] bf16
  PE matmuls on blocked (lhsT must be a single contiguous free dim)

Phase ph covers the contiguous T-range [128*offs, 128*(offs+W)), split
contiguously by partition, so consecutive descriptors read sequential
HBM addresses (full row-buffer locality; the strided-slice mapping
measured ~19 GB/s/engine vs ~21.5 here -- engines are the wall-to-wall
bottleneck).  The Gram is invariant to this time-permutation: all 24
rows + the ones row of a sample use the same mapping, so every product
pairs equal t.

The tail after the last input byte is sem-receipt + last-phase repack +
matmuls + output chain.  The final two phases taper (170, 80) and are
split into repack sub-units so PE starts right after each phase's DMA
sem; ph3 <= ~175 cols keeps its matmuls clear of ph4's sem.  The output
chain (PSUM -> SBUF -> HBM) lives entirely on the scalar engine: the
copy, desc-gen, and DMA run back-to-back with no cross-engine sem hop.

Blocked layout: free = (g, r, c): index = 130*g + 5*r + c, r in 0..25
 (rows 0..23 = data row r = s*6 + t*3 + i, row 24 = ones, row 25 = pad),
 c in 0..4, chunk = 5*g + c.

  per block g one matmul, lhsT = rhs = blocked[:, 130g:130g+125]
      psum_gram[125,125] += lhsT.T @ rhs ; m = 5*r + c
      host: Gamma[ra, rb] = sum_c psum[5*ra+c, 5*rb+c]
"""

import numpy as np
from contextlib import ExitStack

import concourse.bass as bass
from concourse import mybir

F32 = mybir.dt.float32
BF16 = mybir.dt.bfloat16
AFT = mybir.ActivationFunctionType

C = 3
B_LOC = 4
P = 128
PD = 125           # psum partition count (25 rows x 5 chunk-cols)
NROW = 26          # 24 data rows + ones(24) + pad(25)
NDATA = 24
BLK = 5 * NROW     # 130: free elems per chunk-group block


def build_nc(T=128000, widths=(250, 250, 250, 170, 80), n_units=(1, 1, 1, 2, 2),
             dual_ring=True):
    """Build the per-core Bass program. Returns (nc, meta)."""
    TCOLS = T // P
    assert TCOLS * P == T
    widths = tuple(widths)
    NPH = len(widths)
    assert sum(widths) == TCOLS
    offs = [sum(widths[:i]) for i in range(NPH)]
    Gs = [w // 5 for w in widths]
    assert all(w % (5 * nu) == 0 for w, nu in zip(widths, n_units))

    nc = bass.Bass(trn_type="TRN2", target_bir_lowering=False, debug=False)

    sep = nc.dram_tensor("sep", [B_LOC, C, T], F32, kind="ExternalInput").ap()
    src = nc.dram_tensor("src", [B_LOC, C, T], F32, kind="ExternalInput").ap()
    gram_out = nc.dram_tensor("gram", [PD, PD], F32, kind="ExternalOutput").ap()

    # SBUF (one slot per phase, sized to that phase: no reuse, no WAR waits;
    # the slots sum to one full copy of the input + 1/2-size bf16 blocked)
    stg = [nc.alloc_sbuf_tensor(f"stg{i}", [P, NDATA * w], F32).ap()
           for i, w in enumerate(widths)]
    blk = [nc.alloc_sbuf_tensor(f"blk{i}", [P, NROW * w], BF16).ap()
           for i, w in enumerate(widths)]

    out_sb = nc.alloc_sbuf_tensor("out_sb", [PD, PD], F32).ap()

    # PSUM
    ps_g = nc.alloc_psum_tensor("ps_g", [PD, PD], F32).ap()

    def stg3(i):
        return stg[i].rearrange("p (r n) -> p r n", r=NDATA)

    def stg4(i):  # [p, r, g, c]
        return stg[i].rearrange("p (r g c) -> p r g c", r=NDATA, g=Gs[i])

    def blk4(i):  # [p, r, g, c] view of blocked (g, r, c) layout
        return blk[i].rearrange("p (g r c) -> p r g c", g=Gs[i], r=NROW)

    # compute units: (phase, g0, g1); tail phases split so PE starts right
    # after the phase's DMA sem instead of after a full-width repack
    units = []
    for ph in range(NPH):
        gt = Gs[ph] // n_units[ph]
        units += [(ph, u * gt, (u + 1) * gt) for u in range(n_units[ph])]
    NU = len(units)

    # repack row split across the two fast copy engines: while desc-gen still
    # occupies ACT (early phases) give DVE more rows; in the tail split evenly
    def rows_of(ph):
        return ((0, 12), (12, 24)) if ph >= NPH - 2 else ((0, 17), (17, 24))

    with ExitStack() as ctx:
        # one sem per phase: each counts ONLY its own 8 DMAs' engine-level
        # increments (16 each).  A single per-ring counter would be racy:
        # increments are per-SDMA-engine, so a fast engine's incs for phase
        # ph+1 can stand in for a slow engine's missing phase-ph incs.
        st_sems = [ctx.enter_context(nc.semaphore(f"st_sem{i}"))
                   for i in range(NPH)]
        odma_sem = ctx.enter_context(nc.semaphore("odma_sem"))
        rpk_dve = ctx.enter_context(nc.semaphore("rpk_dve"))
        rpk_act = ctx.enter_context(nc.semaphore("rpk_act"))
        rpk_gp = ctx.enter_context(nc.semaphore("rpk_gp"))
        pe_sem = ctx.enter_context(nc.semaphore("pe_sem"))
        block = ctx.enter_context(nc.Block())

        def issue_phase_dmas(eng, ph, samples, ring):
            s3 = stg3(ph)
            for s in samples:
                for t, big in enumerate((sep, src)):
                    r0 = s * 6 + t * 3
                    src_ap = big[s][:, P * offs[ph]:P * (offs[ph] + widths[ph])
                                    ].rearrange("i (p w) -> p i w", p=P)
                    eng.dma_start(out=s3[:, r0:r0 + 3, :], in_=src_ap
                                  ).then_inc(st_sems[ph], 16)

        def wait_phase_data(eng, ph):
            eng.wait_ge(st_sems[ph], 16 * 8)

        @block.sync
        def _(sync: bass.BassEngine):
            # big data split across both HWDGE rings (desc-gen is the issue-rate
            # bottleneck: ~0.7-1.3us per DMA, serialized per ring)
            for ph in range(NPH):
                issue_phase_dmas(sync, ph, (0, 1) if dual_ring else (0, 1, 2, 3),
                                 ring=0)
            # scalar owns the output chain; just hold the block open until done
            sync.wait_ge(odma_sem, 16)

        @block.gpsimd
        def _(gpsimd: bass.BassEngine):
            # ones(row 24) + pad(row 25) once per blocked slot; no repack here
            # (gpsimd copies measured ~6x slower than DVE)
            for i in range(NPH):
                ap1 = blk[i].rearrange("p (g x) -> p g x", g=Gs[i])[
                    :, :, 5 * NDATA:5 * NROW]
                gpsimd.memset(ap1, 1.0).then_inc(rpk_gp, 1)

        def repack_unit(eng, u, which):
            ph, g0, g1 = units[u]
            r0, r1 = rows_of(ph)[which]
            dst = blk4(ph)[:, r0:r1, g0:g1, :]
            src_v = stg4(ph)[:, r0:r1, g0:g1, :]
            return dst, src_v

        @block.scalar
        def _(scalar: bass.BassEngine):
            if dual_ring:
                issue_phase_dmas(scalar, 0, (2, 3), ring=1)
                issue_phase_dmas(scalar, 1, (2, 3), ring=1)
            # per phase: repack this phase's units, then issue phase ph+2's
            # DMAs (ph+1 already issued) so repack(0) is never starved
            last_ph = -1
            for u in range(NU):
                ph = units[u][0]
                if ph != last_ph:
                    wait_phase_data(scalar, ph)
                dst, src_v = repack_unit(scalar, u, 1)
                scalar.activation(dst, src_v, AFT.Copy).then_inc(rpk_act, 1)
                if ph != last_ph and dual_ring and ph + 2 < NPH:
                    issue_phase_dmas(scalar, ph + 2, (2, 3), ring=1)
                last_ph = ph
            # output chain all on this engine: PSUM->SBUF copy, then HWDGE
            # DMA on its own ring (no cross-engine sem hop before desc-gen)
            scalar.wait_ge(pe_sem, NU)
            scalar.activation(out_sb, ps_g, AFT.Copy)
            scalar.dma_start(out=gram_out, in_=out_sb).then_inc(odma_sem, 16)

        @block.vector
        def _(vector: bass.BassEngine):
            last_ph = -1
            for u in range(NU):
                ph = units[u][0]
                if ph != last_ph:
                    wait_phase_data(vector, ph)
                dst, src_v = repack_unit(vector, u, 0)
                vector.tensor_copy(dst, src_v).then_inc(rpk_dve, 1)
                last_ph = ph

        @block.tensor
        def _(tensor: bass.BassEngine):
            nmm = 0
            total_mm = sum(Gs)
            tensor.wait_ge(rpk_gp, NPH)
            for u in range(NU):
                ph, g0, g1 = units[u]
                tensor.wait_ge(rpk_dve, u + 1)
                tensor.wait_ge(rpk_act, u + 1)
                b = blk[ph]
                for g in range(g0, g1):
                    ap = b[:, BLK * g: BLK * g + 125]
                    mm = tensor.matmul(ps_g, ap, ap,
                                       start=(nmm == 0), stop=(nmm == total_mm - 1))
                    nmm += 1
                mm.then_inc(pe_sem, 1)

    meta = dict(T=T, NPH=NPH, dual_ring=dual_ring)
    return nc, meta


# ---------------- host side ----------------

EPS = 1e-8
LAM_SISNR, LAM_DIAR, LAM_EXIST = 1.0, 0.2, 0.2
TSUB = 1000
from itertools import permutations
PERMS = np.array(list(permutations(range(C))), dtype=np.int64)  # [6, 3]


def host_gamma_fp32(g125):
    """g125 [125,125] -> Gamma [25,25]; m = 5*r + c."""
    return np.einsum('acbc->ab', g125.reshape(25, 5, 25, 5).astype(np.float64))


def _clog(x):
    with np.errstate(divide='ignore'):
        return np.maximum(np.log(x), -100.0)


def host_finalize(gammas, exist_probs, num_speakers, diar_probs, labels,
                  T=128000):
    """gammas: list of [25,25] float64 per core. Returns the 5 scalars."""
    B = len(gammas) * B_LOC
    ns = np.asarray(num_speakers).astype(np.int64)

    S = np.zeros((B, C, C), np.float64)
    for core, gam in enumerate(gammas):
        for s in range(B_LOC):
            b = core * B_LOC + s
            e_rows = [s * 6 + i for i in range(3)]
            t_rows = [s * 6 + 3 + j for j in range(3)]
            dot_raw = gam[np.ix_(e_rows, t_rows)]            # [i, j]
            sep_sq = np.array([gam[r, r] for r in e_rows])
            src_sq = np.array([gam[r, r] for r in t_rows])
            sum_sep = gam[e_rows, 24]
            sum_src = gam[t_rows, 24]

            dot = dot_raw - np.outer(sum_sep, sum_src) / T
            est_sq = sep_sq - sum_sep ** 2 / T               # [i]
            tgt_sq = src_sq - sum_src ** 2 / T               # [j]

            alpha = dot / (tgt_sq[None, :] + EPS)
            sig = alpha * alpha * tgt_sq[None, :] + EPS
            noise = est_sq[:, None] - 2.0 * alpha * dot + alpha * alpha * tgt_sq[None, :] + EPS
            S[b] = 10.0 * np.log10(sig / noise)

    # ---- diarization BCE on host (tiny: B x 1000 x 3) ----
    dp = np.asarray(diar_probs, np.float64)
    lb = np.asarray(labels, np.float64)
    T_f = lb.shape[1]
    idx = np.floor(np.arange(TSUB) * (T_f / TSUB)).astype(np.int64)
    tgt_d = lb[:, idx, :]
    logp = _clog(dp)
    log1mp = _clog(1.0 - dp)
    D = -(np.einsum('bti,btj->bij', logp, tgt_d)
          + np.einsum('bti,btj->bij', log1mp, 1.0 - tgt_d)) / TSUB

    n_spk = np.clip(ns, 1, C)
    slot = np.arange(C)
    slot_mask = (slot[None, :] < n_spk[:, None]).astype(np.float64)
    valid = np.all((PERMS[None, :, :] < n_spk[:, None, None])
                   | (slot[None, None, :] >= n_spk[:, None, None]), axis=-1)

    S_perm = S[:, PERMS, slot]                               # [B, 6, 3]
    sisnr_mean = (S_perm * slot_mask[:, None, :]).sum(-1) / n_spk[:, None]
    sisnr_loss_p = np.where(valid, -sisnr_mean, np.inf)
    best = sisnr_loss_p.min(axis=-1)
    loss_sisnr = best.mean()
    mean_sisnr = (-best).mean()

    D_perm = D[:, PERMS, slot]
    diar_p = (D_perm * slot_mask[:, None, :]).sum(-1) / n_spk[:, None]
    loss_diar = np.where(valid, diar_p, np.inf).min(axis=-1).mean()

    ep = np.asarray(exist_probs, np.float64)
    n_ex = np.minimum(ns, C)
    ex_tgt = (np.arange(C + 1)[None, :] < n_ex[:, None]).astype(np.float64)
    bce_ex = -(ex_tgt * _clog(ep) + (1.0 - ex_tgt) * _clog(1.0 - ep))
    loss_exist = bce_ex.mean()

    total = LAM_SISNR * loss_sisnr + LAM_DIAR * loss_diar + LAM_EXIST * loss_exist
    return tuple(np.float32(v) for v in
                 (total, loss_sisnr, loss_diar, loss_exist, mean_sisnr))


def shard_inputs(separated, diar_probs, sources, labels, n_cores=8):
    maps = []
    for c in range(n_cores):
        sl = slice(B_LOC * c, B_LOC * (c + 1))
        maps.append({
            "sep": np.ascontiguousarray(separated[sl], dtype=np.float32),
            "src": np.ascontiguousarray(sources[sl], dtype=np.float32),
        })
    return maps


# ---------------- kernel entry (self-contained) ----------------

N_CORES = 8
_CACHE = {}


def _get_nc():
    if "nc" not in _CACHE:
        _CACHE["nc"] = build_nc(T=128000)[0]
    return _CACHE["nc"]


def kernel(separated, diar_probs, exist_probs, sources, labels, num_speakers):
    """EEND-SS loss on 8 NeuronCores: batch sharded 4 samples/core; device
    computes the big time-axis Gram; host does the tiny PIT/diar/exist math."""
    from concourse.bass_utils import run_bass_kernel_spmd

    separated = np.asarray(separated)
    diar_probs = np.asarray(diar_probs)
    exist_probs = np.asarray(exist_probs)
    sources = np.asarray(sources)
    labels = np.asarray(labels)
    num_speakers = np.asarray(num_speakers)

    nc = _get_nc()
    in_maps = shard_inputs(separated, diar_probs, sources, labels, N_CORES)
    res = run_bass_kernel_spmd(nc, in_maps, list(range(N_CORES)))

    gammas = [host_gamma_fp32(res.results[c]["gram"]) for c in range(N_CORES)]
    return host_finalize(gammas, exist_probs, num_speakers, diar_probs, labels,
                         T=128000)


# revision 28
# speedup vs baseline: 1.1008x; 1.1008x over previous
"""EEND-SS loss device kernel (raw Bass, SPMD over 8 cores).

Device computes, per core (B_LOC=4 samples), the Gram matrix of
[sep rows(12) | src rows(12) | ones] over T, chunk-blocked so the host
extracts all pairwise dots / sums / sq-sums for the SI-SDR PIT loss.
sep+src are 97% of the input bytes (12.3 MB/core); the tiny diarization
BCE (diar_probs 48 KB + labels 384 KB per 4 samples) and existence BCE
are done host-side in numpy -- their scattered-granule DMAs would grind
the SDMA engines for ~30us, longer than the compute they feed.

Pipeline (T split into column-phases of widths[ph] chunks, default
(250, 250, 250, 170, 80)):
  DMA (sequential HBM)        ->  staging[ph]   [128, 24*W] f32
  repack (DVE/ACT row split)  ->  blocked[ph]   [128, 26*W] bf16
  PE matmuls on blocked (lhsT must be a single contiguous free dim)

Phase ph covers the contiguous T-range [128*offs, 128*(offs+W)), split
contiguously by partition, so consecutive descriptors read sequential
HBM addresses (full row-buffer locality; the strided-slice mapping
measured ~19 GB/s/engine vs ~21.5 here -- engines are the wall-to-wall
bottleneck).  The Gram is invariant to this time-permutation: all 24
rows + the ones row of a sample use the same mapping, so every product
pairs equal t.

The tail after the last input byte is sem-receipt + last-phase repack +
matmuls + output chain.  The final two phases taper (170, 80) and are
split into repack sub-units so PE starts right after each phase's DMA
sem; ph3 <= ~175 cols keeps its matmuls clear of ph4's sem.  The output
chain (PSUM -> SBUF -> HBM) lives entirely on the scalar engine: the
copy, desc-gen, and DMA run back-to-back with no cross-engine sem hop.

Blocked layout: free = (g, r, c): index = 130*g + 5*r + c, r in 0..25
 (rows 0..23 = data row r = s*6 + t*3 + i, row 24 = ones, row 25 = pad),
 c in 0..4, chunk = 5*g + c.

  per block g one matmul, lhsT = rhs = blocked[:, 130g:130g+125]
      psum_gram[125,125] += lhsT.T @ rhs ; m = 5*r + c
      host: Gamma[ra, rb] = sum_c psum[5*ra+c, 5*rb+c]
"""

import numpy as np
from contextlib import ExitStack

import concourse.bass as bass
from concourse import mybir

F32 = mybir.dt.float32
BF16 = mybir.dt.bfloat16
AFT = mybir.ActivationFunctionType

C = 3
B_LOC = 4
P = 128
PD = 125           # psum partition count (25 rows x 5 chunk-cols)
NROW = 26          # 24 data rows + ones(24) + pad(25)
NDATA = 24
BLK = 5 * NROW     # 130: free elems per chunk-group block


def build_nc(T=128000, widths=(250, 250, 250, 170, 80), n_units=(1, 1, 1, 2, 2),
             dual_ring=True):
    """Build the per-core Bass program. Returns (nc, meta)."""
    TCOLS = T // P
    assert TCOLS * P == T
    widths = tuple(widths)
    NPH = len(widths)
    assert sum(widths) == TCOLS
    offs = [sum(widths[:i]) for i in range(NPH)]
    Gs = [w // 5 for w in widths]
    assert all(w % (5 * nu) == 0 for w, nu in zip(widths, n_units))

    nc = bass.Bass(trn_type="TRN2", target_bir_lowering=False, debug=False)

    sep = nc.dram_tensor("sep", [B_LOC, C, T], F32, kind="ExternalInput").ap()
    src = nc.dram_tensor("src", [B_LOC, C, T], F32, kind="ExternalInput").ap()
    gram_out = nc.dram_tensor("gram", [PD, PD], F32, kind="ExternalOutput").ap()

    # SBUF (one slot per phase, sized to that phase: no reuse, no WAR waits;
    # the slots sum to one full copy of the input + 1/2-size bf16 blocked)
    stg = [nc.alloc_sbuf_tensor(f"stg{i}", [P, NDATA * w], F32).ap()
           for i, w in enumerate(widths)]
    blk = [nc.alloc_sbuf_tensor(f"blk{i}", [P, NROW * w], BF16).ap()
           for i, w in enumerate(widths)]

    out_sb = nc.alloc_sbuf_tensor("out_sb", [PD, PD], F32).ap()

    # PSUM
    ps_g = nc.alloc_psum_tensor("ps_g", [PD, PD], F32).ap()

    def stg3(i):
        return stg[i].rearrange("p (r n) -> p r n", r=NDATA)

    def stg4(i):  # [p, r, g, c]
        return stg[i].rearrange("p (r g c) -> p r g c", r=NDATA, g=Gs[i])

    def blk4(i):  # [p, r, g, c] view of blocked (g, r, c) layout
        return blk[i].rearrange("p (g r c) -> p r g c", g=Gs[i], r=NROW)

    # compute units: (phase, g0, g1); tail phases split so PE starts right
    # after the phase's DMA sem instead of after a full-width repack
    units = []
    for ph in range(NPH):
        gt = Gs[ph] // n_units[ph]
        units += [(ph, u * gt, (u + 1) * gt) for u in range(n_units[ph])]
    NU = len(units)

    # repack row split across the two fast copy engines: while desc-gen still
    # occupies ACT (early phases) give DVE more rows; in the tail split evenly
    def rows_of(ph):
        return ((0, 15), (15, 24)) if ph >= NPH - 2 else ((0, 17), (17, 24))

    with ExitStack() as ctx:
        # one sem per phase: each counts ONLY its own 8 DMAs' engine-level
        # increments (16 each).  A single per-ring counter would be racy:
        # increments are per-SDMA-engine, so a fast engine's incs for phase
        # ph+1 can stand in for a slow engine's missing phase-ph incs.
        st_sems = [ctx.enter_context(nc.semaphore(f"st_sem{i}"))
                   for i in range(NPH)]
        odma_sem = ctx.enter_context(nc.semaphore("odma_sem"))
        rpk_dve = ctx.enter_context(nc.semaphore("rpk_dve"))
        rpk_act = ctx.enter_context(nc.semaphore("rpk_act"))
        rpk_gp = ctx.enter_context(nc.semaphore("rpk_gp"))
        pe_sem = ctx.enter_context(nc.semaphore("pe_sem"))
        block = ctx.enter_context(nc.Block())

        def issue_phase_dmas(eng, ph, samples, ring):
            s3 = stg3(ph)
            for s in samples:
                for t, big in enumerate((sep, src)):
                    r0 = s * 6 + t * 3
                    src_ap = big[s][:, P * offs[ph]:P * (offs[ph] + widths[ph])
                                    ].rearrange("i (p w) -> p i w", p=P)
                    eng.dma_start(out=s3[:, r0:r0 + 3, :], in_=src_ap
                                  ).then_inc(st_sems[ph], 16)

        def wait_phase_data(eng, ph):
            eng.wait_ge(st_sems[ph], 16 * 8)

        @block.sync
        def _(sync: bass.BassEngine):
            # big data split across both HWDGE rings (desc-gen is the issue-rate
            # bottleneck: ~0.7-1.3us per DMA, serialized per ring)
            for ph in range(NPH):
                issue_phase_dmas(sync, ph, (0, 1) if dual_ring else (0, 1, 2, 3),
                                 ring=0)
            # scalar owns the output chain; just hold the block open until done
            sync.wait_ge(odma_sem, 16)

        @block.gpsimd
        def _(gpsimd: bass.BassEngine):
            # ones(row 24) + pad(row 25) once per blocked slot; no repack here
            # (gpsimd copies measured ~6x slower than DVE)
            for i in range(NPH):
                ap1 = blk[i].rearrange("p (g x) -> p g x", g=Gs[i])[
                    :, :, 5 * NDATA:5 * NROW]
                gpsimd.memset(ap1, 1.0).then_inc(rpk_gp, 1)

        def repack_unit(eng, u, which):
            ph, g0, g1 = units[u]
            r0, r1 = rows_of(ph)[which]
            dst = blk4(ph)[:, r0:r1, g0:g1, :]
            src_v = stg4(ph)[:, r0:r1, g0:g1, :]
            return dst, src_v

        @block.scalar
        def _(scalar: bass.BassEngine):
            if dual_ring:
                issue_phase_dmas(scalar, 0, (2, 3), ring=1)
                issue_phase_dmas(scalar, 1, (2, 3), ring=1)
            # per phase: repack this phase's units, then issue phase ph+2's
            # DMAs (ph+1 already issued) so repack(0) is never starved
            last_ph = -1
            for u in range(NU):
                ph = units[u][0]
                if ph != last_ph:
                    wait_phase_data(scalar, ph)
                dst, src_v = repack_unit(scalar, u, 1)
                scalar.activation(dst, src_v, AFT.Copy).then_inc(rpk_act, 1)
                if ph != last_ph and dual_ring and ph + 2 < NPH:
                    issue_phase_dmas(scalar, ph + 2, (2, 3), ring=1)
                last_ph = ph
            # output chain all on this engine: PSUM->SBUF copy, then HWDGE
            # DMA on its own ring (no cross-engine sem hop before desc-gen)
            scalar.wait_ge(pe_sem, NU)
            scalar.activation(out_sb, ps_g, AFT.Copy)
            scalar.dma_start(out=gram_out, in_=out_sb).then_inc(odma_sem, 16)

        @block.vector
        def _(vector: bass.BassEngine):
            last_ph = -1
            for u in range(NU):
                ph = units[u][0]
                if ph != last_ph:
                    wait_phase_data(vector, ph)
                dst, src_v = repack_unit(vector, u, 0)
                vector.tensor_copy(dst, src_v).then_inc(rpk_dve, 1)
                last_ph = ph

        @block.tensor
        def _(tensor: bass.BassEngine):
            nmm = 0
            total_mm = sum(Gs)
            tensor.wait_ge(rpk_gp, NPH)
            for u in range(NU):
                ph, g0, g1 = units[u]
                tensor.wait_ge(rpk_dve, u + 1)
                tensor.wait_ge(rpk_act, u + 1)
                b = blk[ph]
                for g in range(g0, g1):
                    ap = b[:, BLK * g: BLK * g + 125]
                    mm = tensor.matmul(ps_g, ap, ap,
                                       start=(nmm == 0), stop=(nmm == total_mm - 1))
                    nmm += 1
                mm.then_inc(pe_sem, 1)

    meta = dict(T=T, NPH=NPH, dual_ring=dual_ring)
    return nc, meta


# ---------------- host side ----------------

EPS = 1e-8
LAM_SISNR, LAM_DIAR, LAM_EXIST = 1.0, 0.2, 0.2
TSUB = 1000
from itertools import permutations
PERMS = np.array(list(permutations(range(C))), dtype=np.int64)  # [6, 3]


def host_gamma_fp32(g125):
    """g125 [125,125] -> Gamma [25,25]; m = 5*r + c."""
    return np.einsum('acbc->ab', g125.reshape(25, 5, 25, 5).astype(np.float64))


def _clog(x):
    with np.errstate(divide='ignore'):
        return np.maximum(np.log(x), -100.0)


def host_finalize(gammas, exist_probs, num_speakers, diar_probs, labels,
                  T=128000):
    """gammas: list of [25,25] float64 per core. Returns the 5 scalars."""
    B = len(gammas) * B_LOC
    ns = np.asarray(num_speakers).astype(np.int64)

    S = np.zeros((B, C, C), np.float64)
    for core, gam in enumerate(gammas):
        for s in range(B_LOC):
            b = core * B_LOC + s
            e_rows = [s * 6 + i for i in range(3)]
            t_rows = [s * 6 + 3 + j for j in range(3)]
            dot_raw = gam[np.ix_(e_rows, t_rows)]            # [i, j]
            sep_sq = np.array([gam[r, r] for r in e_rows])
            src_sq = np.array([gam[r, r] for r in t_rows])
            sum_sep = gam[e_rows, 24]
            sum_src = gam[t_rows, 24]

            dot = dot_raw - np.outer(sum_sep, sum_src) / T
            est_sq = sep_sq - sum_sep ** 2 / T               # [i]
            tgt_sq = src_sq - sum_src ** 2 / T               # [j]

            alpha = dot / (tgt_sq[None, :] + EPS)
            sig = alpha * alpha * tgt_sq[None, :] + EPS
            noise = est_sq[:, None] - 2.0 * alpha * dot + alpha * alpha * tgt_sq[None, :] + EPS
            S[b] = 10.0 * np.log10(sig / noise)

    # ---- diarization BCE on host (tiny: B x 1000 x 3) ----
    dp = np.asarray(diar_probs, np.float64)
    lb = np.asarray(labels, np.float64)
    T_f = lb.shape[1]
    idx = np.floor(np.arange(TSUB) * (T_f / TSUB)).astype(np.int64)
    tgt_d = lb[:, idx, :]
    logp = _clog(dp)
    log1mp = _clog(1.0 - dp)
    D = -(np.einsum('bti,btj->bij', logp, tgt_d)
          + np.einsum('bti,btj->bij', log1mp, 1.0 - tgt_d)) / TSUB

    n_spk = np.clip(ns, 1, C)
    slot = np.arange(C)
    slot_mask = (slot[None, :] < n_spk[:, None]).astype(np.float64)
    valid = np.all((PERMS[None, :, :] < n_spk[:, None, None])
                   | (slot[None, None, :] >= n_spk[:, None, None]), axis=-1)

    S_perm = S[:, PERMS, slot]                               # [B, 6, 3]
    sisnr_mean = (S_perm * slot_mask[:, None, :]).sum(-1) / n_spk[:, None]
    sisnr_loss_p = np.where(valid, -sisnr_mean, np.inf)
    best = sisnr_loss_p.min(axis=-1)
    loss_sisnr = best.mean()
    mean_sisnr = (-best).mean()

    D_perm = D[:, PERMS, slot]
    diar_p = (D_perm * slot_mask[:, None, :]).sum(-1) / n_spk[:, None]
    loss_diar = np.where(valid, diar_p, np.inf).min(axis=-1).mean()

    ep = np.asarray(exist_probs, np.float64)
    n_ex = np.minimum(ns, C)
    ex_tgt = (np.arange(C + 1)[None, :] < n_ex[:, None]).astype(np.float64)
    bce_ex = -(ex_tgt * _clog(ep) + (1.0 - ex_tgt) * _clog(1.0 - ep))
    loss_exist = bce_ex.mean()

    total = LAM_SISNR * loss_sisnr + LAM_DIAR * loss_diar + LAM_EXIST * loss_exist
    return tuple(np.float32(v) for v in
                 (total, loss_sisnr, loss_diar, loss_exist, mean_sisnr))


def shard_inputs(separated, diar_probs, sources, labels, n_cores=8):
    maps = []
    for c in range(n_cores):
        sl = slice(B_LOC * c, B_LOC * (c + 1))
        maps.append({
            "sep": np.ascontiguousarray(separated[sl], dtype=np.float32),
            "src": np.ascontiguousarray(sources[sl], dtype=np.float32),
        })
    return maps


# ---------------- kernel entry (self-contained) ----------------

N_CORES = 8
_CACHE = {}


def _get_nc():
    if "nc" not in _CACHE:
        _CACHE["nc"] = build_nc(T=128000)[0]
    return _CACHE["nc"]


def kernel(separated, diar_probs, exist_probs, sources, labels, num_speakers):
    """EEND-SS loss on 8 NeuronCores: batch sharded 4 samples/core; device
    computes the big time-axis Gram; host does the tiny PIT/diar/exist math."""
    from concourse.bass_utils import run_bass_kernel_spmd

    separated = np.asarray(separated)
    diar_probs = np.asarray(diar_probs)
    exist_probs = np.asarray(exist_probs)
    sources = np.asarray(sources)
    labels = np.asarray(labels)
    num_speakers = np.asarray(num_speakers)

    nc = _get_nc()
    in_maps = shard_inputs(separated, diar_probs, sources, labels, N_CORES)
    res = run_bass_kernel_spmd(nc, in_maps, list(range(N_CORES)))

    gammas = [host_gamma_fp32(res.results[c]["gram"]) for c in range(N_CORES)]
    return host_finalize(gammas, exist_probs, num_speakers, diar_probs, labels,
                         T=128000)
